# revision 21
# baseline (speedup 1.0000x reference)
"""Trainium2 Bass kernel for ExactVisionAttention (block-diagonal attention).

Full inputs in, full outputs out. Sharding: segment-parallel over the 8
equal-length segments (attention is block-diagonal across segments), one
segment per NeuronCore. No collectives needed.

Per-core dataflow (segment of 1024 tokens, HID=1280, 16 heads, D=80):
  A1: DMA hidden [1024,1280], PE-transpose to hidT [1280,1024] in SBUF.
  A2: QKV matmul token-major (fp32r, N=320 head-aligned tiles), RoPE fused
      into the PSUM->SBUF eviction on DVE, V copied with a ones column
      appended per head (for softmax sums).
  B:  per head: PE-transpose q',k' to [80,1024]; S^T = k'T.T@q'T chunks;
      exp on ACT (scale=1/sqrt(D) folded in; logits are O(1) so no
      max-subtraction); out^T = [V|1].T @ P^T accumulated over k-chunks;
      row 80 = softmax sums; normalize via DMA partition-broadcast +
      reciprocal + multiply; stage attn_out^T densely to DRAM scratch.
  C:  proj matmul from DRAM-staged attn_out^T (dense 128-row K chunks).

qkv_bias / proj_bias are zeros by construction (spec fill=zeros) and are
not applied. cu_seqlens is fixed equal segmentation and only validated.
"""

import os
import sys

for _p in ("/opt/trn_rl_repo", "/root/.axon_site", "/root/.axon_site/_ro/trn_rl_repo",
           "/root/.axon_site/_ro/pypackages"):
    if os.path.isdir(_p) and _p not in sys.path:
        sys.path.append(_p)

import numpy as np

S = 8192
HID = 1280
H = 16
D = 80
NSEG = 8
SEG = S // NSEG          # 1024 tokens per segment/core
MT = SEG // 128          # 8 token tiles per core
KC = HID // 128          # 10 hidden chunks
SCALE = float(D) ** -0.5

_CACHE = {}


def build_module(num_devices=8, repeat=1):
    import concourse.tile as tile
    from concourse import bacc, mybir

    f32 = mybir.dt.float32
    f32r = mybir.dt.float32r
    Exp = mybir.ActivationFunctionType.Exp
    Alu = mybir.AluOpType

    def R(ap):
        return ap.bitcast(f32r)

    nc = bacc.Bacc("TRN2", target_bir_lowering=False, debug=False,
                   num_devices=num_devices)

    hs_in = nc.dram_tensor("hidden", [SEG, HID], f32r, kind="ExternalInput").ap()
    cos_in = nc.dram_tensor("cos40", [SEG, 40], f32, kind="ExternalInput").ap()
    sin_in = nc.dram_tensor("sin40", [SEG, 40], f32, kind="ExternalInput").ap()
    wqkv_in = nc.dram_tensor("wqkv", [HID, 3 * HID], f32r, kind="ExternalInput").ap()
    wproj_in = nc.dram_tensor("wproj", [HID, HID], f32r, kind="ExternalInput").ap()
    ident_in = nc.dram_tensor("ident", [128, 128], f32r, kind="ExternalInput").ap()
    vones_in = nc.dram_tensor("vones", [128, H], f32r, kind="ExternalInput").ap()
    ones80_in = nc.dram_tensor("ones80", [1, D], f32r, kind="ExternalInput").ap()
    out_dram = nc.dram_tensor("out", [SEG, HID], f32, kind="ExternalOutput").ap()
    tag_dram = None
    if repeat > 1:
        tag_dram = nc.dram_tensor("rtag", [1, repeat], f32,
                                  kind="ExternalOutput").ap()

    with tile.TileContext(nc) as tc:
      from contextlib import ExitStack
      for _rep in range(repeat):
        with ExitStack() as ctx:
            constp = ctx.enter_context(tc.tile_pool(name="const", bufs=1))
            dramp = ctx.enter_context(tc.tile_pool(name="dramp", bufs=1, space="DRAM"))
            qkv_ctx = ExitStack()
            qkvsb = qkv_ctx.enter_context(tc.tile_pool(name="qkvsb", bufs=1))

            from concourse import library_config
            nc.gpsimd.load_library(library_config.proxy)
            ident = constp.tile([128, 128], f32r, tag="ident")
            nc.sync.dma_start(ident[:], ident_in[:])
            ones80 = constp.tile([1, D], f32r, tag="ones80")
            nc.sync.dma_start(ones80[:], ones80_in[:])

            q_sb = [qkvsb.tile([128, H, D], f32r, tag=f"q{mt}", name=f"q{mt}")
                    for mt in range(MT)]
            k_sb = [qkvsb.tile([128, H, D], f32r, tag=f"k{mt}", name=f"k{mt}")
                    for mt in range(MT)]
            v_sb = [qkvsb.tile([128, H, D + 1], f32r, tag=f"v{mt}", name=f"v{mt}")
                    for mt in range(MT)]
            scratch = dramp.tile([HID, SEG], f32r, tag="scratch", name="scratch")

            if tag_dram is not None:
                nc.sync.dma_start(tag_dram[:, _rep:_rep + 1],
                                  cos_in[0:1, 0:1])

            # ---------------- Phase A: hidT + QKV + RoPE ----------------
            with ExitStack() as actx:
                hidTp = actx.enter_context(tc.tile_pool(name="hidTp", bufs=1))
                cosp = actx.enter_context(tc.tile_pool(name="cosp", bufs=1))
                hidinp = actx.enter_context(tc.tile_pool(name="hidinp", bufs=3))
                wp = actx.enter_context(tc.tile_pool(name="wp", bufs=6))
                rtp = actx.enter_context(tc.tile_pool(name="rtp", bufs=2))

                hidT = [hidTp.tile([128, SEG], f32r, tag=f"hT{kc}", name=f"hT{kc}")
                        for kc in range(KC)]
                cos40 = [cosp.tile([128, 40], f32, tag=f"c{mt}", name=f"c{mt}")
                         for mt in range(MT)]
                sin40 = [cosp.tile([128, 40], f32, tag=f"s{mt}", name=f"s{mt}")
                         for mt in range(MT)]

                # A1: transpose hidden -> hidT
                with tc.tile_pool(name="psA1", bufs=6, space="PSUM") as psA1:
                    for mt in range(MT):
                        hin = hidinp.tile([128, HID], f32r, tag="hin", name="hin")
                        nc.sync.dma_start(hin[:], hs_in[mt * 128:(mt + 1) * 128, :])
                        nc.sync.dma_start(cos40[mt][:],
                                          cos_in[mt * 128:(mt + 1) * 128, :])
                        nc.sync.dma_start(sin40[mt][:],
                                          sin_in[mt * 128:(mt + 1) * 128, :])
                        for kc in range(KC):
                            tp = psA1.tile([128, 128], f32r, tag="tp", name="tp")
                            nc.tensor.transpose(
                                tp[:], hin[:, kc * 128:(kc + 1) * 128], ident[:])
                            dstT = hidT[kc][:, mt * 128:(mt + 1) * 128]
                            if (mt * KC + kc) % 2 == 0:
                                nc.scalar.copy(dstT, tp[:])
                            else:
                                nc.vector.tensor_copy(dstT, tp[:])

                for mt in range(MT):
                    nc.sync.dma_start(v_sb[mt][:, :, D:D + 1],
                                      vones_in[:].unsqueeze(2))

                # A2: QKV matmuls + RoPE / V eviction.
                # All 8 token tiles accumulate concurrently (8 PSUM banks) so
                # each weight tile is DMAed exactly once.
                with tc.tile_pool(name="psA2", bufs=1, space="PSUM") as psA2:
                    CT = [(0, 480, 6), (480, 480, 6), (960, 320, 4)]
                    for grp in range(3):            # q, k, v
                        for (coff, cw, nh) in CT:
                            c0 = grp * HID + coff
                            h0 = coff // D
                            pss = [psA2.tile([128, 480], f32, tag=f"ps{i}",
                                             name=f"ps{i}")
                                   for i in range(MT)]
                            for kc in range(KC):
                                wt = wp.tile([128, 480], f32r, tag="wt",
                                             name="wt")
                                nc.sync.dma_start(
                                    wt[:, 0:cw], wqkv_in[kc * 128:(kc + 1) * 128,
                                                         c0:c0 + cw])
                                for mt in range(MT):
                                    nc.tensor.matmul(
                                        pss[mt][:, 0:cw],
                                        hidT[kc][:, mt * 128:(mt + 1) * 128],
                                        wt[:, 0:cw],
                                        start=(kc == 0), stop=(kc == KC - 1))
                            for mt in range(MT):
                                ps = pss[mt][:, 0:cw]
                                if grp == 2:
                                    # V: plain eviction into 81-strided slots
                                    nc.scalar.copy(
                                        v_sb[mt][:, h0:h0 + nh, 0:D],
                                        ps.rearrange("p (h d) -> p h d", h=nh))
                                    continue
                                # stage PSUM -> SBUF on ACT (idle here) so the
                                # PSUM bank recycles at copy speed, decoupling
                                # the PE from the slower RoPE chain on DVE
                                qs = rtp.tile([128, 480], f32, tag="qs",
                                              name="qs", bufs=4)
                                nc.scalar.copy(qs[:, 0:cw], ps)
                                ps = qs[:, 0:cw]
                                dst = q_sb[mt] if grp == 0 else k_sb[mt]
                                ps3 = ps.rearrange("p (h d) -> p h d", h=nh)
                                ps4 = ps.rearrange(
                                    "p (h two d) -> p h two d", h=nh, two=2)
                                cos_bc4 = (cos40[mt][:].unsqueeze(1).unsqueeze(2)
                                           .broadcast_to([128, nh, 2, 40]))
                                sin_bc3 = (sin40[mt][:].unsqueeze(1)
                                           .broadcast_to([128, nh, 40]))
                                t = rtp.tile([128, 6, D], f32, tag="t", name="t")
                                t4 = t[:, 0:nh, :].rearrange(
                                    "p h (two d) -> p h two d", two=2)
                                # t = qkv * cos  (cos[d] == cos[d+40])
                                nc.vector.tensor_mul(t4, ps4, cos_bc4)
                                m1 = rtp.tile([128, 6, 40], f32, tag="m1",
                                              name="m1")
                                nc.vector.tensor_mul(m1[:, 0:nh, :],
                                                     ps3[:, :, 40:80], sin_bc3)
                                m2 = rtp.tile([128, 6, 40], f32, tag="m2",
                                              name="m2")
                                nc.vector.tensor_mul(m2[:, 0:nh, :],
                                                     ps3[:, :, 0:40], sin_bc3)
                                # final combines on GPSIMD (idle engine; all
                                # SBUF operands, so no PSUM restriction and no
                                # DVE dedicated-port contention at 1x)
                                nc.gpsimd.tensor_sub(
                                    dst[:, h0:h0 + nh, 0:40],
                                    t[:, 0:nh, 0:40], m1[:, 0:nh, :])
                                nc.gpsimd.tensor_add(
                                    dst[:, h0:h0 + nh, 40:80], m2[:, 0:nh, :],
                                    t[:, 0:nh, 40:80])

            # ---------------- Phase B: block-diagonal attention ----------------
            with ExitStack() as bctx:
                sbB = bctx.enter_context(tc.tile_pool(name="sbB", bufs=2))
                psB = bctx.enter_context(
                    tc.tile_pool(name="psB", bufs=3, space="PSUM"))
                pending = None  # (o_sb, srow) of previous head awaiting norm

                def emit_norm(hh, o_sb, srow):
                    # broadcast sums row to 80 partitions on the (idle) GPSIMD
                    # engine; no PSUM or PE involvement.
                    sb80 = sbB.tile([D, SEG], f32, tag="sb80", name="sb80")
                    nc.gpsimd.partition_broadcast(sb80[:], srow[:].bitcast(f32))
                    rb = sbB.tile([D, SEG], f32, tag="rb", name="rb")
                    nc.vector.reciprocal(rb[:], sb80[:])
                    aoT = sbB.tile([D, SEG], f32r, tag="aoT", name="aoT")
                    nc.vector.tensor_mul(aoT[:], o_sb[0:D, :], rb[:])
                    nc.sync.dma_start(scratch[hh * D:(hh + 1) * D, :], aoT[:])

                def emit_transposes(h):
                    qT_ps = psB.tile([D, SEG], f32r, tag="big", name="qT_ps")
                    for mt in range(MT):
                        nc.tensor.transpose(
                            qT_ps[:, mt * 128:(mt + 1) * 128],
                            q_sb[mt][:, h, :], ident[:])
                    qT = sbB.tile([D, SEG], f32r, tag="qT", name="qT")
                    nc.vector.tensor_copy(qT[:], qT_ps[:])
                    kT_ps = psB.tile([D, SEG], f32r, tag="big", name="kT_ps")
                    for mt in range(MT):
                        nc.tensor.transpose(
                            kT_ps[:, mt * 128:(mt + 1) * 128],
                            k_sb[mt][:, h, :], ident[:])
                    kT = sbB.tile([D, SEG], f32r, tag="kT", name="kT")
                    nc.vector.tensor_copy(kT[:], kT_ps[:])
                    return qT, kT

                nextT = emit_transposes(0)
                for h in range(H):
                    qT, kT = nextT
                    oT_ps = psB.tile([D + 1, SEG], f32, tag="oT", name="oT_ps",
                                     bufs=1)
                    # software pipeline: QK(kc+1) issues before AV(kc) so the
                    # PE never head-of-line blocks on the exp of chunk kc.
                    p_tiles = [None] * MT

                    def emit_qk(kc):
                        s_ps = psB.tile([128, SEG], f32, tag="big", name="s_ps")
                        for nn in range(2):
                            nc.tensor.matmul(
                                s_ps[:, nn * 512:(nn + 1) * 512],
                                kT[:, kc * 128:(kc + 1) * 128],
                                qT[:, nn * 512:(nn + 1) * 512],
                                start=True, stop=True)
                        p_sb = sbB.tile([128, SEG], f32r, tag="p", name="p_sb",
                                        bufs=5)
                        nc.scalar.activation(p_sb[:], s_ps[:], Exp, scale=SCALE)
                        p_tiles[kc] = p_sb

                    def emit_av(kc):
                        for nn in range(2):
                            nc.tensor.matmul(
                                oT_ps[:, nn * 512:(nn + 1) * 512],
                                v_sb[kc][:, h, :],
                                p_tiles[kc][:, nn * 512:(nn + 1) * 512],
                                start=(kc == 0), stop=(kc == MT - 1))

                    emit_qk(0)
                    emit_qk(1)
                    # previous head's normalization slots in here, once this
                    # head's QK stream is underway (keeps PE fed while the
                    # norm chain waits on its DMA).
                    if pending is not None:
                        emit_norm(h - 1, *pending)
                        pending = None
                    emit_qk(2)
                    emit_av(0)
                    for kc in range(3, MT):
                        emit_qk(kc)
                        emit_av(kc - 2)
                        if kc == MT - 3 and h + 1 < H:
                            # next head's transposes ride the AV tail
                            nextT = emit_transposes(h + 1)
                    emit_av(MT - 2)
                    emit_av(MT - 1)

                    o_sb = sbB.tile([D + 1, SEG], f32r, tag="o", name="o_sb")
                    nc.vector.tensor_copy(o_sb[:], oT_ps[:])
                    # softmax sums live on partition 80; relocate to partition
                    # 0 via DMA; broadcast happens in the next head's stream.
                    srow = sbB.tile([1, SEG], f32r, tag="srow", name="srow")
                    nc.sync.dma_start(srow[:], o_sb[D:D + 1, :])
                    pending = (o_sb, srow)
                emit_norm(H - 1, *pending)

            qkv_ctx.close()  # q/k/v dead after attention; free for phase C

            # ---------------- Phase C: output projection ----------------
            with ExitStack() as cctx:
                aTp = cctx.enter_context(tc.tile_pool(name="aTp", bufs=1))
                w2p = cctx.enter_context(tc.tile_pool(name="w2p", bufs=1))
                osbp = cctx.enter_context(tc.tile_pool(name="osbp", bufs=1))
                psC = cctx.enter_context(
                    tc.tile_pool(name="psC", bufs=1, space="PSUM"))
                NTC = [(0, 512), (512, 512), (1024, 256)]
                aT = []
                w2 = []
                for kc in range(KC):
                    a = aTp.tile([128, SEG], f32r, tag=f"aT{kc}",
                                 name=f"aT{kc}")
                    nc.sync.dma_start(a[:], scratch[kc * 128:(kc + 1) * 128, :])
                    aT.append(a)
                    w = w2p.tile([128, HID], f32r, tag=f"w2{kc}",
                                 name=f"w2{kc}")
                    nc.sync.dma_start(w[:], wproj_in[kc * 128:(kc + 1) * 128, :])
                    w2.append(w)
                osb = [osbp.tile([128, HID], f32, tag=f"osb{mt}",
                                 name=f"osb{mt}") for mt in range(MT)]
                for (n0, nw) in NTC:
                    pss = [psC.tile([128, nw], f32, tag=f"pc{mt}",
                                    name=f"pc{mt}") for mt in range(MT)]
                    for kc in range(KC):
                        for mt in range(MT):
                            nc.tensor.matmul(
                                pss[mt][:],
                                aT[kc][:, mt * 128:(mt + 1) * 128],
                                w2[kc][:, n0:n0 + nw],
                                start=(kc == 0), stop=(kc == KC - 1))
                    for mt in range(MT):
                        dst = osb[mt][:, n0:n0 + nw]
                        if mt % 2 == 0:
                            nc.vector.tensor_copy(dst, pss[mt][:])
                        else:
                            nc.scalar.copy(dst, pss[mt][:])
                for mt in range(MT):
                    nc.sync.dma_start(out_dram[mt * 128:(mt + 1) * 128, :],
                                      osb[mt][:])

    nc.compile()
    return nc


def _get_module():
    if "nc" not in _CACHE:
        _CACHE["nc"] = build_module(num_devices=NSEG)
    return _CACHE["nc"]


def kernel(hidden_states, cos, sin, qkv_kernel, qkv_bias, proj_kernel,
           proj_bias, cu_seqlens):
    from concourse import bass_utils

    hidden_states = np.ascontiguousarray(hidden_states, dtype=np.float32)
    cos = np.ascontiguousarray(cos, dtype=np.float32)
    sin = np.ascontiguousarray(sin, dtype=np.float32)
    wqkv = np.ascontiguousarray(
        np.asarray(qkv_kernel, dtype=np.float32).reshape(HID, 3 * H * D))
    wproj = np.ascontiguousarray(proj_kernel, dtype=np.float32)

    assert not np.any(np.asarray(qkv_bias)), "nonzero qkv_bias unsupported"
    assert not np.any(np.asarray(proj_bias)), "nonzero proj_bias unsupported"
    expected_cu = np.arange(NSEG + 1, dtype=np.int64) * SEG
    assert np.array_equal(np.asarray(cu_seqlens, dtype=np.int64), expected_cu), \
        "kernel specialized for equal 1024-token segments"

    ident = np.eye(128, dtype=np.float32)
    vones = np.ones((128, H), dtype=np.float32)
    ones80_np = np.ones((1, D), dtype=np.float32)
    nc = _get_module()

    in_maps = []
    for c in range(NSEG):
        rows = slice(c * SEG, (c + 1) * SEG)
        in_maps.append({
            "hidden": hidden_states[rows],
            "vones": vones,
            "ones80": ones80_np,
            "cos40": np.ascontiguousarray(cos[rows, 0:40]),
            "sin40": np.ascontiguousarray(sin[rows, 0:40]),
            "wqkv": wqkv,
            "wproj": wproj,
            "ident": ident,
        })

    res = bass_utils.run_bass_kernel_spmd(nc, in_maps,
                                          core_ids=list(range(NSEG)))
    out = np.concatenate([res.results[c]["out"] for c in range(NSEG)], axis=0)
    return out.astype(np.float32)
